# revision 22
# baseline (speedup 1.0000x reference)
"""Trainium2 Bass kernel for nn_CrossLayerAttention_309237645906.

Reference computation (B=2, SQ=SK=2048, H=2048, NH=16, HD=128, fp32):
    q = hidden @ w_q.T + b_q                     -> [B, NH, SQ, HD]
    scores = mask + scale * q @ k                (k given as [B*NH, HD, SK])
    probs = softmax(scores)                      (fp32)
    out = (probs @ v)                            -> [B, SQ, H]
    y = out @ w_proj.T + b_proj

Sharding: 8 cores = (batch b = c//4) x (512-row query slice, r = 512*(c%4)).
Each core computes its 512 rows of the final output end-to-end; outputs are
disjoint row slices so no cross-core reduction is needed.

Per-core layout is "transposed" (T-layout): everything that streams through
the tensor engine keeps the contraction dim on partitions, so no on-device
transposes are needed anywhere:
    qT[o, i]      = (w_qT stationary) @ (xT moving)        o-tile == head
    scoresT[j, i] = (k_h tile stationary) @ qT_h            per (head, j-tile)
    t = scores + maskT/scale   (one fused DVE op; mask is data => any mask ok)
    p = exp(scale * t)         (ScalarE; no max-subtraction: |scaled scores|
                                is O(10) here so fp32 exp cannot overflow)
    outT_h[d, i] += (v_h tile stationary) @ p   ;   Z[1, i] += (ones) @ p
    attnT_h = outT_h * (1/Z broadcast via rank-1 PE matmul)
    y[i, o] = (attnT stationary) @ w_projT moving + b_proj

Matmuls run as float32r (full-rate fp32 mode, ~2e-4 scale-relative error);
set _mm_dt=float32 in kernel() for exact-but-4x-slower matmuls.
"""

import sys

sys.path.insert(0, "/opt/trn_rl_repo")

import numpy as np

import concourse.bacc as bacc
import concourse.bass as bass
import concourse.mybir as mybir
import concourse.tile as tile
from concourse.bass_utils import run_bass_kernel_spmd

F32 = mybir.dt.float32
F32R = mybir.dt.float32r
BF16 = mybir.dt.bfloat16

B, SQ, SK, H, NH = 2, 2048, 2048, 2048, 16
HD = H // NH  # 128
ROWS = 512            # query rows per core
NCORES = 8
KT = H // 128         # 16 contraction tiles for the projections
JT = SK // 128        # 16 key tiles
IT = ROWS // 128      # 4 query 128-tiles per core
SCALE = 1.0 / float(np.sqrt(HD))
MULT = mybir.AluOpType.mult
ADD = mybir.AluOpType.add
EXP = mybir.ActivationFunctionType.Exp
IDENT = mybir.ActivationFunctionType.Identity


def build_kernel(mm_dt=F32R, mask_dt=BF16, cfg=None, causal=False):
    """Build the per-core Bass program.

    mm_dt:   dtype tag for matmul operands (F32R = full-rate, F32 = exact)
    mask_dt: dtype of the on-chip additive mask (BF16 is exact for the
             causal 0/-1e9 mask; use F32 for arbitrary masks)
    """
    cfg = {**dict(kv=2, tp=4, pp=4, p1w=2, scb=4, zpb=1, opb=1, GS=1,
                  wpp=8 if mask_dt == BF16 else 4),
           **(cfg or {})}
    GS = cfg["GS"]
    nc = bacc.Bacc()

    xT = nc.dram_tensor("xT", [H, ROWS], mm_dt, kind="ExternalInput")
    wqT = nc.dram_tensor("wqT", [H, H], mm_dt, kind="ExternalInput")
    bq = nc.dram_tensor("bq", [H, 1], F32, kind="ExternalInput")
    key = nc.dram_tensor("key", [NH, HD, SK], mm_dt, kind="ExternalInput")
    value = nc.dram_tensor("value", [NH, SK, HD], mm_dt, kind="ExternalInput")
    maskT = nc.dram_tensor("maskT", [SK, ROWS], mask_dt, kind="ExternalInput")
    wpT = nc.dram_tensor("wpT", [H, H], mm_dt, kind="ExternalInput")
    bpB = nc.dram_tensor("bpB", [128, H], F32, kind="ExternalInput")
    onesd = nc.dram_tensor("onesd", [128, 1], mm_dt, kind="ExternalInput")
    ones1d = nc.dram_tensor("ones1d", [1, 128], mm_dt, kind="ExternalInput")
    Y = nc.dram_tensor("Y", [ROWS, H], F32, kind="ExternalOutput")

    with tile.TileContext(nc) as tc:
        with tc.tile_pool(name="res", bufs=1) as res:
            # ---- resident tiles (live across phases) ----
            qT_all = res.tile([128, KT, ROWS], mm_dt)
            attnT_all = res.tile([128, NH, ROWS], mm_dt)
            maskT_all = res.tile([128, JT, ROWS], mask_dt)
            bq_all = res.tile([128, KT, 1], F32)
            nc.sync.dma_start(bq_all, bq[:, :].rearrange("(t p) x -> p t x", p=128))
            bpB_all = res.tile([128, H], F32)
            nc.sync.dma_start(bpB_all, bpB[:, :])
            ones_sb = res.tile([128, 1], mm_dt)
            nc.sync.dma_start(ones_sb, onesd[:, :])
            ones1_sb = res.tile([1, 128], mm_dt)
            nc.sync.dma_start(ones1_sb, ones1d[:, :])

            # pools that should overlap across phases (released LIFO)
            wpp = tc.alloc_tile_pool(name="wpp", bufs=cfg["wpp"])
            kv = tc.alloc_tile_pool(name="kv", bufs=cfg["kv"])
            tp = tc.alloc_tile_pool(name="tp", bufs=cfg["tp"])
            pp = tc.alloc_tile_pool(name="pp", bufs=cfg["pp"])
            ps_s = tc.alloc_tile_pool(name="ps_s", bufs=cfg["scb"], space="PSUM")
            ps_z = tc.alloc_tile_pool(name="ps_z", bufs=cfg["zpb"], space="PSUM")
            ps_o = tc.alloc_tile_pool(name="ps_o", bufs=cfg["opb"], space="PSUM")

            # ---- phase 1: q projection (per o-tile == head) ----
            with tc.tile_pool(name="p1", bufs=1) as p1, \
                 tc.tile_pool(name="p1w", bufs=cfg["p1w"]) as p1w, \
                 tc.tile_pool(name="ps_q", bufs=2, space="PSUM") as ps_q:
                xT_all = p1.tile([128, KT, ROWS], mm_dt)
                xT_ap = xT[:, :].rearrange("(t p) i -> p t i", p=128)
                for k in range(KT):
                    nc.sync.dma_start(xT_all[:, k, :], xT_ap[:, k, :])
                wqT_ap = wqT[:, :].rearrange("(a p) o -> p a o", p=128)
                for t in range(KT):
                    w_sb = p1w.tile([128, KT, 128], mm_dt, tag="wq")
                    nc.sync.dma_start(w_sb[:, :KT // 2, :],
                                      wqT_ap[:, :KT // 2, 128 * t:128 * (t + 1)])
                    nc.sync.dma_start(w_sb[:, KT // 2:, :],
                                      wqT_ap[:, KT // 2:, 128 * t:128 * (t + 1)])
                    psq = ps_q.tile([128, ROWS], F32, tag="psq")
                    for k in range(KT):
                        nc.tensor.matmul(psq, w_sb[:, k, :], xT_all[:, k, :],
                                         start=(k == 0), stop=(k == KT - 1))
                    nc.scalar.activation(qT_all[:, t, :], psq, IDENT,
                                         bias=bq_all[:, t, :])

            # ---- phase 2: attention per head ----
            sm = tc.alloc_tile_pool(name="sm", bufs=2)
            maskT_ap = maskT[:, :].rearrange("(t p) i -> p t i", p=128)
            for j in range(JT):
                nc.sync.dma_start(maskT_all[:, j, :], maskT_ap[:, j, :])
            EA = 8  # causal: padded j-tile extent for the low 256 rows
            for h in range(NH):
                k_sbs, v_sbs = [], []
                for hf in range(2):
                    k_sb = kv.tile([128, JT // 2, 128], mm_dt, tag="k",
                                   name=f"k{h}_{hf}")
                    nc.sync.dma_start(
                        k_sb, key[h, :, 1024 * hf:1024 * (hf + 1)]
                        .rearrange("d (a j) -> d a j", j=128))
                    v_sb = kv.tile([128, JT // 2, 128], mm_dt, tag="v",
                                   name=f"v{h}_{hf}")
                    nc.sync.dma_start(
                        v_sb, value[h, 1024 * hf:1024 * (hf + 1), :]
                        .rearrange("(a p) d -> p a d", p=128))
                    k_sbs.append(k_sb)
                    v_sbs.append(v_sb)

                zp = ps_z.tile([1, ROWS], F32, tag="z")
                op = ps_o.tile([128, ROWS], F32, tag="o")
                pend = []  # software pipeline: consume p one group late

                def consume(gp, p_tile):
                    for uu in range(p_tile.shape[1]):
                        jtc = GS * gp + uu
                        wide = not causal or jtc < EA
                        o_dst = op if wide else op[:, 256:]
                        z_dst = zp if wide else zp[:, 256:]
                        nc.tensor.matmul(o_dst, v_sbs[jtc // 8][:, jtc % 8, :],
                                         p_tile[:, uu, :],
                                         start=(jtc == 0), stop=(jtc == JT - 1),
                                         skip_group_check=causal)
                        nc.tensor.matmul(z_dst, ones_sb, p_tile[:, uu, :],
                                         start=(jtc == 0), stop=(jtc == JT - 1),
                                         skip_group_check=causal)

                for g in range(JT // GS):
                    wide = not causal or GS * g < EA
                    W = ROWS if wide else ROWS // 2
                    sc = ps_s.tile([128, GS * W], F32, tag="s", name=f"sc{h}_{g}")
                    t_sb = tp.tile([128, GS, W], F32, tag="t", name=f"t{h}_{g}")
                    for u in range(GS):
                        jt = GS * g + u
                        q_src = qT_all[:, h, :] if wide else qT_all[:, h, 256:]
                        m_src = (maskT_all[:, jt, :] if wide
                                 else maskT_all[:, jt, 256:])
                        nc.tensor.matmul(sc[:, W * u:W * (u + 1)],
                                         k_sbs[jt // 8][:, jt % 8, :],
                                         q_src, start=True, stop=True)
                        nc.vector.scalar_tensor_tensor(
                            t_sb[:, u, :], sc[:, W * u:W * (u + 1)],
                            1.0, m_src, MULT, ADD)
                    p_sb = pp.tile([128, GS, W], mm_dt, tag="p", name=f"p{h}_{g}")
                    nc.scalar.activation(p_sb, t_sb, EXP, scale=SCALE)
                    pend.append((g, p_sb))
                    if len(pend) > 1:
                        consume(*pend.pop(0))
                while pend:
                    consume(*pend.pop(0))

                # normalize: attnT_h = op * (1/Z), 1/Z broadcast via PE matmul
                rc = sm.tile([1, ROWS], mm_dt, tag="rc")
                with nc.allow_low_precision(reason="f32r reciprocal storage"):
                    nc.vector.reciprocal(rc, zp)
                bc = ps_s.tile([128, ROWS], F32, tag="s")
                nc.tensor.matmul(bc, ones1_sb, rc, start=True, stop=True)
                rb = sm.tile([128, ROWS], F32, tag="rb")
                nc.scalar.copy(rb, bc)
                nc.vector.tensor_tensor(attnT_all[:, h, :], op, rb, op=MULT)

            sm.release()
            ps_o.release()
            ps_z.release()
            ps_s.release()
            pp.release()
            tp.release()
            kv.release()

            # ---- phase 3: output projection ----
            with tc.tile_pool(name="ypo", bufs=2) as ypo, \
                 tc.tile_pool(name="ps_y", bufs=4, space="PSUM") as ps_y:
                wpT_ap = wpT[:, :].rearrange("(a p) o -> p a o", p=128)
                for half in range(2):
                    o0 = 1024 * half
                    psys = []
                    for it in range(IT):
                        psy = ps_y.tile([128, 1024], F32, tag="y",
                                        name=f"psy{half}_{it}")
                        psys.append(psy)
                    for k in range(KT):
                        wp_sb = wpp.tile([128, 1024], mm_dt, tag="wp")
                        nc.sync.dma_start(wp_sb, wpT_ap[:, k, o0:o0 + 1024])
                        for it in range(IT):
                            att = attnT_all[:, k, 128 * it:128 * (it + 1)]
                            for nb in range(2):
                                nc.tensor.matmul(
                                    psys[it][:, 512 * nb:512 * (nb + 1)],
                                    att, wp_sb[:, 512 * nb:512 * (nb + 1)],
                                    start=(k == 0), stop=(k == KT - 1))
                    for it in range(IT):
                        y_sb = ypo.tile([128, 1024], F32, tag="ysb")
                        nc.vector.tensor_tensor(y_sb, psys[it],
                                                bpB_all[:, o0:o0 + 1024], op=ADD)
                        nc.sync.dma_start(
                            Y[128 * it:128 * (it + 1), o0:o0 + 1024], y_sb)
            wpp.release()

    nc.compile()
    return nc


_CACHE = {}


def _get_nc(mm_dt, mask_dt, causal):
    ck = (str(mm_dt), str(mask_dt), causal)
    if ck not in _CACHE:
        _CACHE[ck] = build_kernel(mm_dt, mask_dt, causal=causal)
    return _CACHE[ck]


def _is_causal(attention_mask):
    """True if the mask is exactly the standard causal additive mask."""
    m = attention_mask
    if m.shape != (B, 1, SQ, SK):
        return False
    m0 = np.asarray(m[0, 0])
    tri = np.tril(np.ones((SQ, SK), dtype=bool))
    ref = np.where(tri, np.float32(0.0), np.float32(-1e9))
    if not np.array_equal(m0, ref):
        return False
    for b in range(1, B):
        if not np.array_equal(np.asarray(m[b, 0]), m0):
            return False
    return True


def kernel(hidden_states, key, value, attention_mask, w_q, b_q, w_proj, b_proj,
           _mm_dt=F32R, _trace=False):
    hidden_states = np.asarray(hidden_states)
    key = np.asarray(key)
    value = np.asarray(value)
    attention_mask = np.asarray(attention_mask)
    w_q = np.asarray(w_q)
    b_q = np.asarray(b_q)
    w_proj = np.asarray(w_proj)
    b_proj = np.asarray(b_proj)

    import ml_dtypes
    causal = _is_causal(attention_mask)
    mask_dt = BF16 if causal else F32
    mask_np = ml_dtypes.bfloat16 if causal else np.float32

    nc = _get_nc(_mm_dt, mask_dt, causal)

    wqT = np.ascontiguousarray(w_q.T)
    wpT = np.ascontiguousarray(w_proj.T)
    bq2 = np.ascontiguousarray(b_q[:, None]).astype(np.float32)
    bpB = np.ascontiguousarray(
        np.broadcast_to(b_proj[None, :], (128, H))).astype(np.float32)
    key_b = [np.ascontiguousarray(key[b * NH:(b + 1) * NH]) for b in range(B)]
    val_b = [np.ascontiguousarray(value[b]) for b in range(B)]
    inv_scale = np.float32(1.0 / SCALE)

    def core_rows(c):
        b = c // 4
        s = c % 4
        if causal:
            return b, np.r_[256 * s:256 * s + 256, 256 * (7 - s):256 * (7 - s) + 256]
        return b, np.arange(ROWS * s, ROWS * s + ROWS)

    in_maps = []
    for c in range(NCORES):
        b, rows = core_rows(c)
        xT_c = np.ascontiguousarray(hidden_states[b, rows, :].T)
        maskT_c = np.ascontiguousarray(
            (attention_mask[b, 0, rows, :].T * inv_scale).astype(mask_np))
        in_maps.append(dict(
            xT=xT_c, wqT=wqT, bq=bq2, key=key_b[b], value=val_b[b],
            maskT=maskT_c, wpT=wpT, bpB=bpB,
            onesd=np.ones((128, 1), dtype=np.float32),
            ones1d=np.ones((1, 128), dtype=np.float32),
        ))

    kw = {}
    if _trace:
        kw = dict(trace=True, trace_cores=list(range(NCORES)), stitch_traces=False)
    res = run_bass_kernel_spmd(nc, in_maps, core_ids=list(range(NCORES)), **kw)
    if _trace:
        kernel._last_result = res

    out = np.empty((B, SQ, H), dtype=np.float32)
    for c in range(NCORES):
        b, rows = core_rows(c)
        out[b, rows, :] = res.results[c]["Y"]
    return out


if __name__ == "__main__":
    pass
